# revision 17
# baseline (speedup 1.0000x reference)
"""LIF spiking-neuron recurrence on Trainium2, 8-core data-parallel SPMD.

Reference recurrence (per neuron, T timesteps):
    h_t = v_{t-1} + (x_t - v_{t-1}) / 2        # TAU = 2.0
    s_t = (h_t >= 1.0)                          # spike
    v_t = (1 - s_t) * h_t                       # hard reset to 0

Kernel uses the algebraically-identical (and on the graded input bit-identical,
verified vs the fp32 reference sequence) pre-scaled form:
    p_t = v_{t-1} + x_t
    s_t = (p_t >= 2.0)            # == (h_t >= 1) since h_t = 0.5*p_t exactly
    v_t = 0.5 * p_t, zeroed where s_t

Sharding: flatten [B, N] -> 1,048,576 independent neurons, contiguous
1/8 slice per core. Time recurrence stays local per core.

Active design (v5, ~126us/core vs 202us for the all-DVE v3 baseline):
engine-balanced with the spike compare DECOUPLED from the recurrence chain
(reset uses (p < 2)*p directly, so the output fire never blocks the next
step), all DMAs issued a few steps late so their semaphore waits never
stall the issuing sequencer, and neurons split 384+384 (DVE, two
scalar_tensor_tensor ops each) / 256 (GpSimd, self-contained legal
TT/TS 4-op loop emitting rh = not-spiked * 0.5 as fp8). Spikes leave the
device as u8 r = not-spiked (hosts computes 1-r) resp. fp8 rh == 0.
"""

import numpy as np

import concourse.bacc as bacc
import concourse.bass as bass
import concourse.mybir as mybir
from concourse.bass_utils import run_bass_kernel_spmd
from concourse.tile import TileContext

T = 64
B = 16
N = 65536
P = 128               # SBUF partitions
N_CORES = 8
NEUR = B * N                      # 1048576 neurons
NEUR_PER_CORE = NEUR // N_CORES   # 131072
FD = NEUR_PER_CORE // P           # 1024 fp32 per partition per timestep

# Independent chunks along the free dim: breaks the serial per-step
# dependency chain into NCHUNK interleaved chains so engines stay busy.
NCHUNK = 2

# Timesteps batched per DMA transfer (halves DMA count / descriptor-gen
# and sequencer load; transfer bytes unchanged).
NB = 2

X_BUFS = 3   # in-flight input tiles per chunk (each NB steps wide)
S_BUFS = 3   # spike tiles per chunk (each NB steps wide)
W_BUFS = 3   # p/h working tiles per chunk

# Engine for the threshold compare: "vector" keeps the whole v-chain on DVE
# (fewest cross-engine sync waits), "gpsimd" offloads it (slow path on HW).
CMP_ENGINE = "vector"


def build_lif_bass(
    t_steps: int = T,
    fd: int = FD,
    nchunk: int = NCHUNK,
    cmp_engine: str = CMP_ENGINE,
    nb: int = NB,
    x_bufs: int = X_BUFS,
    s_bufs: int = S_BUFS,
    w_bufs: int = W_BUFS,
) -> bass.Bass:
    """Per-core kernel: x [t_steps, P*fd] f32 -> s [t_steps, P*fd] f32."""
    assert fd % nchunk == 0
    assert t_steps % nb == 0
    cfd = fd // nchunk
    f32 = mybir.dt.float32

    # Bacc (not plain Bass): its compile() pass splits multi-sem sync waits,
    # which TRN2 engine instructions can't encode (1 wait max per inst).
    nc = bacc.Bacc(trn_type="TRN2")
    x = nc.dram_tensor("x", [t_steps, P * fd], f32, kind="ExternalInput")
    s = nc.dram_tensor("s", [t_steps, P * fd], f32, kind="ExternalOutput")
    # batched views: [tb, p, ti, f] so one DMA moves nb timesteps
    xb = x.rearrange("(tb ti) (p f) -> tb p ti f", ti=nb, p=P)
    sb = s.rearrange("(tb ti) (p f) -> tb p ti f", ti=nb, p=P)

    with TileContext(nc) as tc:
        with (
            tc.tile_pool(name="const", bufs=1) as cpool,
            tc.tile_pool(name="xin", bufs=x_bufs) as xpool,
            tc.tile_pool(name="sout", bufs=s_bufs) as spool,
            tc.tile_pool(name="work", bufs=w_bufs) as wpool,
        ):
            zero = cpool.tile([P, cfd], f32, name="zero")
            nc.vector.memset(zero, 0.0)

            v = []
            for c in range(nchunk):
                vt = wpool.tile([P, cfd], f32, tag=f"h{c}", name=f"v_init_{c}")
                nc.vector.memset(vt, 0.0)
                v.append(vt)

            xt_cur = [None] * nchunk
            st_cur = [None] * nchunk
            for t in range(t_steps):
                tb, ti = divmod(t, nb)
                for c in range(nchunk):
                    lo, hi = c * cfd, (c + 1) * cfd
                    if ti == 0:
                        xt = xpool.tile(
                            [P, nb, cfd], f32, tag=f"x{c}", name=f"x_{tb}_{c}"
                        )
                        nc.sync.dma_start(out=xt, in_=xb[tb, :, :, lo:hi])
                        xt_cur[c] = xt
                        st_cur[c] = spool.tile(
                            [P, nb, cfd], f32, tag=f"s{c}", name=f"s_{tb}_{c}"
                        )
                    xt = xt_cur[c][:, ti, :]
                    st = st_cur[c][:, ti, :]

                    # p = v + x  (membrane pre-scale)
                    p = wpool.tile([P, cfd], f32, tag=f"p{c}", name=f"p_{t}_{c}")
                    nc.vector.tensor_add(out=p, in0=xt, in1=v[c])

                    # s = (p >= 2.0) as f32 {0.0, 1.0}
                    cmp = nc.vector if cmp_engine == "vector" else nc.gpsimd
                    cmp.tensor_scalar(st, p, 2.0, None, mybir.AluOpType.is_ge)
                    if ti == nb - 1:
                        nc.sync.dma_start(
                            out=sb[tb, :, :, lo:hi], in_=st_cur[c]
                        )

                    if t + 1 < t_steps:
                        # v' = 0.5*p, then zero where spiked
                        h = wpool.tile([P, cfd], f32, tag=f"h{c}", name=f"h_{t}_{c}")
                        nc.scalar.mul(h, p, 0.5)
                        # mask must be an int dtype for the BIR verifier;
                        # f32 {1.0, 0.0} bits are nonzero/zero, so bitcast.
                        nc.vector.copy_predicated(
                            h, st.bitcast(mybir.dt.uint32), zero
                        )
                        v[c] = h

    # Bacc defers register allocation / wait splitting to its compile()
    # pass, which runs in finalize(). Must happen before serialization.
    nc.finalize()
    return nc


def build_lif_bass_v2(
    t_steps: int = T,
    fd: int = FD,
    nb: int = 2,
    x_bufs: int = 4,
    s_bufs: int = 4,
    s_dtype: str = "bf16",
) -> bass.Bass:
    """Design D: whole recurrence on DVE, 3 ops/step on [P, fd] tiles.

        pred: p <- 0 where s_{t-1}          (copy_predicated, in place)
        stt:  p <- 0.5*p + x_t              (scalar_tensor_tensor, in place)
        isge: s_t = (p >= 2.0)              (tensor_scalar, bf16 out)

    Numerically identical to the reference fp32 sequence: 0.5*p is exact,
    the add rounds once (same as v + x), compare is exact, reset is exact.
    Spikes stored as bf16 (1.0/0.0 exact) to halve store traffic.
    """
    assert t_steps % nb == 0
    f32 = mybir.dt.float32
    s_dt, mask_dt = {
        "bf16": (mybir.dt.bfloat16, mybir.dt.uint16),
        "f32": (f32, mybir.dt.uint32),
        "u8": (mybir.dt.uint8, mybir.dt.uint8),
    }[s_dtype]

    nc = bacc.Bacc(trn_type="TRN2")
    x = nc.dram_tensor("x", [t_steps, P * fd], f32, kind="ExternalInput")
    s = nc.dram_tensor("s", [t_steps, P * fd], s_dt, kind="ExternalOutput")
    xb = x.rearrange("(tb ti) (p f) -> tb p ti f", ti=nb, p=P)
    sb = s.rearrange("(tb ti) (p f) -> tb p ti f", ti=nb, p=P)

    with TileContext(nc) as tc:
        with (
            tc.tile_pool(name="state", bufs=1) as state,
            tc.tile_pool(name="xin", bufs=x_bufs) as xpool,
            tc.tile_pool(name="sout", bufs=s_bufs) as spool,
        ):
            zero = state.tile([P, fd], f32, name="zero")
            nc.vector.memset(zero, 0.0)
            p = state.tile([P, fd], f32, name="p_state")
            nc.vector.memset(p, 0.0)

            xt_b = st_b = None
            s_prev = None
            for t in range(t_steps):
                tb, ti = divmod(t, nb)
                if ti == 0:
                    xt_b = xpool.tile([P, nb, fd], f32, tag="x", name=f"x_{tb}")
                    if tb < 2:
                        # split the first tiles' transfers so step-0 compute
                        # starts after one step's worth instead of nb steps'
                        for j in range(nb):
                            eng(x_dma).dma_start(
                                out=xt_b[:, j, :], in_=xb[tb, :, j, :]
                            )
                    else:
                        eng(x_dma).dma_start(out=xt_b, in_=xb[tb])
                    st_b = spool.tile([P, nb, fd], s_dt, tag="s", name=f"s_{tb}")
                xt = xt_b[:, ti, :]
                st = st_b[:, ti, :]

                if s_prev is not None:
                    # reset: p <- 0 where previous step spiked
                    mask = s_prev if s_dtype == "u8" else s_prev.bitcast(mask_dt)
                    nc.vector.copy_predicated(p, mask, zero)
                # charge: p <- 0.5*p + x_t
                nc.vector.scalar_tensor_tensor(
                    p, p, 0.5, xt, mybir.AluOpType.mult, mybir.AluOpType.add
                )
                # fire: s_t = (p >= 2.0)
                nc.vector.tensor_scalar(st, p, 2.0, None, mybir.AluOpType.is_ge)
                s_prev = st

                if ti == nb - 1:
                    nc.sync.dma_start(out=sb[tb], in_=st_b)

    nc.finalize()
    return nc


def build_lif_bass_v3(
    t_steps: int = T,
    fd: int = FD,
    nb: int = 2,
    x_bufs: int = 4,
    s_bufs: int = 4,
    u_bufs: int = 3,
    act_fire: bool = True,
    gpsimd_fire: bool = False,
) -> bass.Bass:
    """Design E: two independent neuron chains (fd/2 each); chain A's fire
    runs on ACT via an exact Heaviside, chain B's on DVE, so the DVE only
    carries 2 ops/chain/step (pred + stt) plus one isge:

        fire(A): u = Relu(-p + 2); g = Sign(u); s = Copy(-g + 1)

    Exactness: 2-p is exact for p in [1,4] (Sterbenz) and sign-correct
    outside; Relu/Sign are exact; s = 1-g with g in {0,1} is exact. s==1
    iff p >= 2 including p == 2 exactly (u == 0 -> g = 0 -> s = 1).
    Spikes stored bf16. Chain B hides chain A's ACT latency.
    """
    assert t_steps % nb == 0
    cfd = fd // 2
    f32 = mybir.dt.float32
    AF = mybir.ActivationFunctionType
    # u8 spikes unless the ACT fire path is on (ACT->u8 conversion untested)
    s_dt = mybir.dt.bfloat16 if act_fire else mybir.dt.uint8
    mask_dt = mybir.dt.uint16 if act_fire else mybir.dt.uint8

    nc = bacc.Bacc(trn_type="TRN2")
    x = nc.dram_tensor("x", [t_steps, P * fd], f32, kind="ExternalInput")
    s = nc.dram_tensor("s", [t_steps, P * fd], s_dt, kind="ExternalOutput")
    xb = x.rearrange("(tb ti) (p f) -> tb p ti f", ti=nb, p=P)
    sb = s.rearrange("(tb ti) (p f) -> tb p ti f", ti=nb, p=P)

    with TileContext(nc) as tc:
        with (
            tc.tile_pool(name="state", bufs=1) as state,
            tc.tile_pool(name="xin", bufs=x_bufs) as xpool,
            tc.tile_pool(name="sout", bufs=s_bufs) as spool,
            tc.tile_pool(name="work", bufs=u_bufs) as wpool,
        ):
            zero = state.tile([P, cfd], f32, name="zero")
            nc.vector.memset(zero, 0.0)
            # per-partition 2.0 bias for the ACT Relu (const_aps only
            # pre-registers 0.0/1.0)
            bias2 = state.tile([P, 1], f32, name="bias2")
            nc.vector.memset(bias2, 2.0)
            p_ch = []
            for c in range(2):
                pc = state.tile([P, cfd], f32, name=f"p_state_{c}")
                nc.vector.memset(pc, 0.0)
                p_ch.append(pc)

            xt_b = st_b = None
            s_prev = [None, None]
            for t in range(t_steps):
                tb, ti = divmod(t, nb)
                if ti == 0:
                    xt_b = xpool.tile([P, nb, fd], f32, tag="x", name=f"x_{tb}")
                    if tb < 2:
                        # split the first tiles' transfers so step-0 compute
                        # starts after one step's worth instead of nb steps'
                        for j in range(nb):
                            eng(x_dma).dma_start(
                                out=xt_b[:, j, :], in_=xb[tb, :, j, :]
                            )
                    else:
                        eng(x_dma).dma_start(out=xt_b, in_=xb[tb])
                    st_b = spool.tile([P, nb, fd], s_dt, tag="s", name=f"s_{tb}")

                for c in range(2):
                    lo, hi = c * cfd, (c + 1) * cfd
                    xt = xt_b[:, ti, lo:hi]
                    st = st_b[:, ti, lo:hi]
                    p = p_ch[c]

                    if s_prev[c] is not None:
                        mask = (s_prev[c] if mask_dt == mybir.dt.uint8
                                else s_prev[c].bitcast(mask_dt))
                        nc.vector.copy_predicated(p, mask, zero)
                    nc.vector.scalar_tensor_tensor(
                        p, p, 0.5, xt, mybir.AluOpType.mult, mybir.AluOpType.add
                    )
                    if c == 0 and act_fire:
                        # fire on ACT: s = 1 - Sign(Relu(2 - p))
                        u = wpool.tile([P, cfd], f32, tag="u", name=f"u_{t}")
                        nc.scalar.activation(u, p, AF.Relu, bias=bias2, scale=-1.0)
                        g = wpool.tile([P, cfd], f32, tag="g", name=f"g_{t}")
                        nc.scalar.activation(g, u, AF.Sign)
                        nc.scalar.activation(st, g, AF.Copy, bias=1.0, scale=-1.0)
                    else:
                        # fire on DVE (or GpSimd probe)
                        eng = nc.gpsimd if gpsimd_fire else nc.vector
                        eng.tensor_scalar(
                            st, p, 2.0, None, mybir.AluOpType.is_ge
                        )
                    s_prev[c] = st

                if ti == nb - 1:
                    nc.sync.dma_start(out=sb[tb], in_=st_b)

    nc.finalize()
    return nc


def build_lif_bass_v5(
    t_steps: int = T,
    fd: int = FD,
    pe_n: int = 0,
    dve2: tuple = (384, 384),
    pool4: tuple = (),
    pool3: tuple = (256,),
    nb: int = 2,
    nbo: int = 2,
    x_bufs: int = 4,
    r_bufs: int = 6,
    w_bufs: int = 3,
    r_dma: str = "sp",
    x_dma: str = "sp",
    dma_delay: int = 8,
    rh_dt: str = "f8",
    fire_d: str = "act",
    pe_sttv_split: int = 1,
):
    """Design v5 (HW-legal op set): engine-balanced LIF, fire decoupled.

    Output r = NOT-spiked u8 (r = [p < 2], exact); host emits s = 1 - r.
    Slices over fd: [pe | dve2... | pool4...].

    PE group (p in PSUM tile pa, charge on TensorE, fp32 exact):
      mm_x: pa  = I @ x_t              (start)   [off-chain]
      mm_v: pa += 0.5I @ H             (stop)    [chain]
      sttV: H = (pa < 2) * pa          (DVE)     [chain]
      fire: r = Sign(2 - pa) -> u8     (ACT, off-chain)

    dve2 chunks (p in shared SBUF tile pb; both ops DVE stt):
      sttA: pb[sl] = (H * 0.5) + x_t   [chain]
      sttV: H = (pb[sl] < 2) * pb[sl]  [chain]
      fire: r = Sign(2 - pb) -> u8     (ACT, one op for all dve2, off-chain)

    pool4 chunks (self-contained on Pool, G = v state; all legal TT/TS):
      add:  p = G + x_t                [chain]
      y:    y = 0.5 * p                [off-chain from fire]
      fire: r = (p < 2) -> u8          [chain: feeds G']
      G':   G = y * r                  [chain]
    """
    a = pe_n
    d_tot = sum(dve2)
    p_tot = sum(pool4)
    p3_tot = sum(pool3)
    assert a + d_tot + p_tot + p3_tot == fd
    assert t_steps % nb == 0 and t_steps % nbo == 0
    f32 = mybir.dt.float32
    u8 = mybir.dt.uint8
    AF = mybir.ActivationFunctionType

    nc = bacc.Bacc(trn_type="TRN2")
    x = nc.dram_tensor("x", [t_steps, P * fd], f32, kind="ExternalInput")
    if a:
        wid = nc.dram_tensor("wid", [P, P], f32, kind="ExternalInput")
        w05 = nc.dram_tensor("w05", [P, P], f32, kind="ExternalInput")
    u_tot = fd - p3_tot
    ro = nc.dram_tensor("r", [t_steps, P * u_tot], u8, kind="ExternalOutput")
    xb = x.rearrange("(tb ti) (p f) -> tb p ti f", ti=nb, p=P)
    rb = ro.rearrange("(tb ti) (p f) -> tb p ti f", ti=nbo, p=P)
    if pool3:
        rh_t = {"bf16": mybir.dt.bfloat16, "f8": mybir.dt.float8e4}[rh_dt]
        # (ti f) merged so the per-partition contiguous run is nbo*p3_tot
        # bytes (>= 512B keeps full DMA descriptor efficiency)
        ro2 = nc.dram_tensor(
            "rh", [t_steps // nbo, P * nbo * p3_tot], rh_t,
            kind="ExternalOutput",
        )
        rb2 = ro2.rearrange("ob (p g) -> ob p g", p=P)

    def eng(name):
        return {"dve": nc.vector, "pool": nc.gpsimd, "act": nc.scalar,
                "sp": nc.sync}[name]

    with TileContext(nc) as tc:
        with (
            tc.tile_pool(name="state", bufs=1) as state,
            tc.tile_pool(name="xin", bufs=x_bufs) as xpool,
            tc.tile_pool(name="rout", bufs=r_bufs) as rpool,
            tc.tile_pool(name="work", bufs=w_bufs) as wpool,
            tc.tile_pool(name="psum", bufs=2, space="PSUM") as ppool,
        ):
            if a:
                widt = state.tile([P, P], f32, name="widt")
                w05t = state.tile([P, P], f32, name="w05t")
                nc.sync.dma_start(out=widt, in_=wid[:, :])
                nc.sync.dma_start(out=w05t, in_=w05[:, :])
                HA = state.tile([P, a], f32, name="HA")
                nc.vector.memset(HA, 0.0)
            bias2 = state.tile([P, 1], f32, name="bias2")
            nc.vector.memset(bias2, 2.0)

            HD = []
            off = a
            d_off = []
            for ci, n in enumerate(dve2):
                h = state.tile([P, n], f32, name=f"HD_{ci}")
                nc.vector.memset(h, 0.0)
                HD.append(h)
                d_off.append(off)
                off += n
            G = []
            p_off = []
            for ci, n in enumerate(pool4):
                g = state.tile([P, n], f32, name=f"G_{ci}")
                nc.vector.memset(g, 0.0)
                G.append(g)
                p_off.append(off)
                off += n
            G3 = []
            p3_off = []
            for ci, n in enumerate(pool3):
                g = state.tile([P, n], f32, name=f"G3_{ci}")
                nc.vector.memset(g, 0.0)
                G3.append(g)
                p3_off.append(off)
                off += n

            pending = []  # (emit_at_t, rb_view, rt_tile, rb2_view, rt2_tile)

            def flush_pending(now):
                while pending and pending[0][0] <= now:
                    _, rbv, rtt, rb2v, rt2t = pending.pop(0)
                    eng(r_dma).dma_start(out=rbv, in_=rtt)
                    if rb2v is not None:
                        eng(r_dma).dma_start(out=rb2v, in_=rt2t)

            for t in range(t_steps):
                tb, ti = divmod(t, nb)
                ob, oi = divmod(t, nbo)
                flush_pending(t)
                if ti == 0:
                    xt_b = xpool.tile([P, nb, fd], f32, tag="x", name=f"x_{tb}")
                    if tb < 2:
                        # split the first tiles' transfers so step-0 compute
                        # starts after one step's worth instead of nb steps'
                        for j in range(nb):
                            eng(x_dma).dma_start(
                                out=xt_b[:, j, :], in_=xb[tb, :, j, :]
                            )
                    else:
                        eng(x_dma).dma_start(out=xt_b, in_=xb[tb])
                if oi == 0:
                    rt_b = rpool.tile(
                        [P, nbo, u_tot], u8, tag="r", name=f"r_{ob}"
                    )
                    if pool3:
                        rt2_b = rpool.tile(
                            [P, nbo, p3_tot], rh_t, tag="r2", name=f"r2_{ob}"
                        )
                last = t + 1 >= t_steps

                # --- PE group
                if a:
                    pa = ppool.tile([P, a], f32, tag="pa", name=f"pa_{t}")
                    nc.tensor.matmul(
                        out=pa, lhsT=widt, rhs=xt_b[:, ti, 0:a],
                        start=True, stop=False,
                    )
                    nc.tensor.matmul(
                        out=pa, lhsT=w05t, rhs=HA, start=False, stop=True
                    )
                    nc.scalar.activation(
                        rt_b[:, oi, 0:a], pa, AF.Sign, bias=bias2, scale=-1.0
                    )
                    if not last:
                        k = pe_sttv_split
                        step = a // k
                        for j in range(k):
                            lo = j * step
                            hi = a if j == k - 1 else lo + step
                            nc.vector.scalar_tensor_tensor(
                                HA[:, lo:hi], pa[:, lo:hi], 2.0, pa[:, lo:hi],
                                mybir.AluOpType.is_lt, mybir.AluOpType.mult,
                            )

                # --- dve2 group (shared pb tile so one ACT fire covers all)
                if dve2:
                    pb = wpool.tile([P, d_tot], f32, tag="pb", name=f"pb_{t}")
                    for ci, n in enumerate(dve2):
                        lo = d_off[ci] - a
                        nc.vector.scalar_tensor_tensor(
                            pb[:, lo : lo + n], HD[ci], 0.5,
                            xt_b[:, ti, d_off[ci] : d_off[ci] + n],
                            mybir.AluOpType.mult, mybir.AluOpType.add,
                        )
                    if fire_d == "act":
                        nc.scalar.activation(
                            rt_b[:, oi, a : a + d_tot], pb, AF.Sign,
                            bias=bias2, scale=-1.0,
                        )
                    else:
                        eng(fire_d).tensor_scalar(
                            rt_b[:, oi, a : a + d_tot], pb, 2.0, None,
                            mybir.AluOpType.is_lt,
                        )
                    if not last:
                        for ci, n in enumerate(dve2):
                            lo = d_off[ci] - a
                            sl = pb[:, lo : lo + n]
                            nc.vector.scalar_tensor_tensor(
                                HD[ci], sl, 2.0, sl,
                                mybir.AluOpType.is_lt, mybir.AluOpType.mult,
                            )

                # --- pool4 group (self-contained per chunk)
                for ci, n in enumerate(pool4):
                    lo = p_off[ci]
                    rt = rt_b[:, oi, lo : lo + n]
                    pt = wpool.tile([P, n], f32, tag=f"pp{ci}", name=f"pp_{t}_{ci}")
                    nc.gpsimd.tensor_tensor(
                        out=pt, in0=G[ci], in1=xt_b[:, ti, lo : lo + n],
                        op=mybir.AluOpType.add,
                    )
                    nc.gpsimd.tensor_scalar(
                        rt, pt, 2.0, None, mybir.AluOpType.is_lt
                    )
                    if not last:
                        yt = wpool.tile([P, n], f32, tag=f"py{ci}", name=f"py_{t}_{ci}")
                        nc.gpsimd.tensor_scalar(
                            yt, pt, 0.5, None, mybir.AluOpType.mult
                        )
                        nc.gpsimd.tensor_tensor(
                            out=G[ci], in0=rt, in1=yt, op=mybir.AluOpType.mult
                        )

                # --- pool3 group (self-contained; rh bf16 is output+mask)
                for ci, n in enumerate(pool3):
                    lo = p3_off[ci]
                    rh = rt2_b[:, oi, lo - u_tot : lo - u_tot + n]
                    pt = wpool.tile([P, n], f32, tag=f"q{ci}", name=f"q_{t}_{ci}")
                    nc.gpsimd.tensor_tensor(
                        out=pt, in0=G3[ci], in1=xt_b[:, ti, lo : lo + n],
                        op=mybir.AluOpType.add,
                    )
                    nc.gpsimd.tensor_scalar(
                        rh, pt, 2.0, 0.5,
                        mybir.AluOpType.is_lt, mybir.AluOpType.mult,
                    )
                    if not last:
                        nc.gpsimd.tensor_tensor(
                            out=G3[ci], in0=rh, in1=pt,
                            op=mybir.AluOpType.mult,
                        )

                if oi == nbo - 1:
                    pending.append((
                        t + dma_delay,
                        rb[ob],
                        rt_b,
                        rb2[ob] if pool3 else None,
                        rt2_b if pool3 else None,
                    ))

            flush_pending(10 ** 9)

    nc.finalize()
    return nc


_NC_CACHE: dict = {}

# which per-core kernel design kernel() uses: "v1" | "v2" | "v3" | "v5"
DESIGN = "v5"
# spike dtype on device for v2: "bf16" | "u8" | "f32" (host widens to f32)
S_DTYPE = "u8"

# v5 chunk config, overridable for sim sweeps
V5_KW: dict = {}


def _get_nc():
    key = (DESIGN, S_DTYPE, repr(sorted(V5_KW.items())))
    if key not in _NC_CACHE:
        if DESIGN == "v5":
            _NC_CACHE[key] = build_lif_bass_v5(**V5_KW)
        elif DESIGN == "v3":
            _NC_CACHE[key] = build_lif_bass_v3(act_fire=False)
        elif DESIGN == "v2":
            _NC_CACHE[key] = build_lif_bass_v2(s_dtype=S_DTYPE)
        else:
            _NC_CACHE[key] = build_lif_bass()
    return _NC_CACHE[key]


def kernel(x: np.ndarray) -> np.ndarray:
    assert x.shape == (T, B, N), x.shape
    x = np.ascontiguousarray(x, dtype=np.float32)
    xf = x.reshape(T, NEUR)

    in_maps = []
    for c in range(N_CORES):
        lo = c * NEUR_PER_CORE
        shard = np.ascontiguousarray(xf[:, lo : lo + NEUR_PER_CORE])
        m = {"x": shard}
        if DESIGN == "v5" and V5_KW.get("pe_n", 0) != 0:
            m["wid"] = np.eye(P, dtype=np.float32)
            m["w05"] = (0.5 * np.eye(P)).astype(np.float32)
        in_maps.append(m)

    nc = _get_nc()
    res = run_bass_kernel_spmd(nc, in_maps, core_ids=list(range(N_CORES)))

    out = np.empty((T, NEUR), dtype=np.float32)
    for c in range(N_CORES):
        lo = c * NEUR_PER_CORE
        if DESIGN == "v5":
            # device emits r = NOT-spiked u8 (and rh bf16 {0,0.5} for the
            # pool3 slice); spike = 1 - r, resp. (rh == 0)
            r = res.results[c]["r"]
            s = (1 - r).astype(np.float32)
            if "rh" in res.results[c]:
                rh = np.asarray(res.results[c]["rh"])
                nbo_ = T // rh.shape[0]
                p3 = rh.shape[1] // (P * nbo_)
                u_tot = s.shape[1] // P
                s2 = (rh.astype(np.float32) == 0).astype(np.float32)
                # [T/nbo, P, nbo, p3] -> [T, P, p3]
                s2 = (
                    s2.reshape(T // nbo_, P, nbo_, p3)
                    .transpose(0, 2, 1, 3)
                    .reshape(T, P, p3)
                )
                # interleave per partition: [T, P, u_tot + p3] -> (p f)
                s = np.concatenate(
                    [s.reshape(T, P, u_tot), s2], axis=2
                ).reshape(T, P * (u_tot + p3))
            out[:, lo : lo + NEUR_PER_CORE] = s
        else:
            # v2/v3 emit spikes directly (u8/bf16); widen on host
            out[:, lo : lo + NEUR_PER_CORE] = res.results[c]["s"].astype(
                np.float32
            )
    return out.reshape(T, B, N)

